# revision 24
# baseline (speedup 1.0000x reference)
"""Trainium2 Bass kernel for K[c,h,l] = sum_n W[c,h,n] * cos(Lambda_im[n] * l).

Shapes (hardcoded): W (1, 512, 4096) f32, Lambda_im (4096,) f32, L = 8192.
Output: (1, 512, 8192) f32.

Strategy: shard L across the 8 cores (1024 columns each). Each core
generates its slice of the cosine-Vandermonde matrix S[n, l] on-chip and
contracts it with W on the PE array, accumulating fp32 in all 8 PSUM
banks over the 4096-long n axis. The contraction is mixed-precision:
the leading 24 chunks run as fp16 matmuls, the last 8 chunks run as
e4m3 DoubleRow matmuls (K=256 per MM at ~2x rate; rhs pairs are stored
interleaved so each 16-bit SBUF word feeds two PE cells). A full-fp8
product has ~3.4% rel-L2 error; the beta=8/32 hybrid scales that by
sqrt(beta) to the measured 1.69%, inside the 2e-2 budget. W is
prescaled by 2^15 on the host (sigma ~ 8, inside e4m3/fp16 normal
range); the PSUM->SBUF copy descales by 2^-15 and the output returns to
HBM as fp16 (halves the drain DMA; adds ~1e-4 error).

Angle pipeline, per core c, chunk q (n = 128q..128q+127), j = 0..1023:
    f_n    = Lambda_im[n] / (2*pi)                       (host, f32)
    base_n = mod(Lambda_im[n]*(1024c)/(2*pi) + .25, 1)   (host f64 -> f32)
    r      = y - round(y),  y = f_n*j + base_n           (one fused custom
             DVE op, FRAC_AFFINE_ANT: the 2^23 add/sub RNE round trick,
             |r| <= 0.5 exactly)
    S[n,j] = sin(r * SIN_SCALE) = cos(Lambda_im[n]*l)    (ACT Sin, fused
             over 2 chunks; fp16 out for the fp16 chunks, e4m3 out for
             the DoubleRow chunks; SIN_SCALE is one ulp under 2*pi so
             the argument stays inside Sin's [-pi, pi])
    out   += Wt_chunk.T @ S_chunk                        (PE, 8 psum banks)

The four stages (DMA+FRAC | Sin | ext | matmuls) are software-pipelined
by hand across superchunks so the strict-FIFO engine queues never
head-of-line block on a cross-engine dependency.

v2 optimizations over the 73.8us baseline (measured ~43us in the same
device state; ~1.7x):
  - dma_fuse=6: weight DMAs fused into 2 large transfers per rep
    (plus one fp8 transfer), alternating the two physical HWDGE rings
    (sync/scalar). 21 small per-superchunk DMAs measured 85.8us of
    pure DMA (each ~2us fixed completion latency + poor small-transfer
    efficiency); fused groups measure 10-15us and hide fully under
    compute.
  - ext: Chebyshev column extension. Only cols [0,768) of each fp16
    chunk are generated via FRAC+Sin; cols [768,1024) come from one
    fused DVE op S[m] = 2cos(256*lam)*S[m-256] - S[m-512] (MULSUB_ANT,
    per-partition scalar = host-precomputed 2cos(256*lam)). Cuts both
    DVE range-reduction and ACT Sin volume by 25% for fp16 chunks at
    ~1e-3 extra noise. fp8 chunks are fully generated (ext on e4m3
    inputs would amplify quantization noise ~sqrt(6)x); their FRAC is
    contiguous and Sin writes the pair-interleaved e4m3 directly via a
    strided output AP.
  - nsup8=5: 10 of 32 contraction chunks in e4m3 DoubleRow (err
    3.38%*sqrt(10/32) = 1.884e-2 measured, inside the 2e-2 budget).
  - unroll=4: four reps per hardware-loop iteration, amortizing the
    For_i all-engine back-edge barrier (2-6us) and pipeline fill.
Measured ablations (same throttled device state, per rep): DMA 10.1us,
sgen (DVE+ACT) 42.8us, PE 46.0us, full 58.7us.
"""

import os

os.environ.setdefault("MYCRO_LOCAL_CACHE", "1")
# no NTFF hook in this container; never let a stray BASS_TRACE break the run
os.environ.setdefault("BASS_NEVER_TRACE", "1")

from contextlib import ExitStack

import ml_dtypes
import numpy as np

import concourse.tile as tile
from concourse import bacc, mybir
from concourse.bass_utils import run_bass_kernel_spmd

N_CORES = 8
H = 512
N = 4096
L_FULL = 8192
P = 128
F = L_FULL // N_CORES  # 1024 columns of L per core
NCH = N // P  # 32 contraction chunks
SUP = 2  # chunks fused per ACT/STT pass
NSUP = NCH // SUP
HT = H // P  # 4 output row tiles
NHALF = 2  # two 512-wide moving halves per 1024 columns

# fp8 hybrid: the last NSUP8 superchunks of the contraction run as
# e4m3 DoubleRow matmuls (K=256 per MM, 2x PE rate). Rel-L2 error of a
# full-fp8 product is ~3.4% (e4m3 has 3 mantissa bits); running a
# beta=8/32 fraction in fp8 scales that by sqrt(beta) -> ~1.7%, inside
# the 2e-2 budget. W is pre-scaled by 2^15 on the host (sigma ~ 8,
# comfortably inside e4m3/fp16 normal range) and the PSUM->SBUF copy
# descales by 2^-15.
NSUP8 = 5
NSUP16 = NSUP - NSUP8
N16 = NSUP16 * SUP * P  # leading contraction rows in fp16
W_PRESCALE = float(2.0**15)
W_DESCALE = float(2.0**-15)

F32 = mybir.dt.float32
F16 = mybir.dt.float16
F8 = mybir.dt.float8e4

MAGIC = float(2**23)
# sin argument window: scale one ulp under 2*pi so |r|<=0.5 maps inside
# the ScalarE Sin domain [-pi, pi].
SIN_SCALE = float(np.nextafter(np.float32(2 * np.pi), np.float32(0)))

_compiled = {}

_FRAC_OP = None

_MULSUB_OP = None


def _mulsub_op():
    """Fused DVE op: out = in0*s0[p] - in1 (Chebyshev column extension)."""
    global _MULSUB_OP
    if _MULSUB_OP is not None:
        return _MULSUB_OP
    from concourse import dve_ops
    from concourse.dve_spec import Spec, Src0, Src1, C0, lower, _has_src1
    from concourse.dve_uop import DveOpSpec

    name = "MULSUB_ANT"
    for existing in dve_ops.OPS:
        if existing.name == name:
            _MULSUB_OP = existing
            return existing

    body = Src0 * C0 - Src1

    def reference(in0, in1, s0, s1, imm2):
        return (
            in0.astype(np.float32) * np.float32(s0) - in1.astype(np.float32)
        ).astype(np.float32)

    spec = Spec(body=body, reference=reference)
    row = dve_ops._CUSTOM_DVE_ROW_BASE + len(dve_ops.OPS)
    assert row < 0x20, "custom-DVE row overflow"
    dve_ops._SUB_OPCODE_FOR_NAME[name] = row
    shas = {}
    for ver in ("v3", "v4"):
        s = DveOpSpec(
            name=name,
            opcode=row,
            uops=lower(spec, ver=ver),
            rd1_en=_has_src1(spec),
        )
        shas[ver] = s.sha(ver)
    op = dve_ops.DveOp(name, spec, subdim=False, uops_sha=shas)
    dve_ops.OPS.append(op)
    dve_ops.CUSTOM_DVE_SPECS[name] = spec
    _MULSUB_OP = op
    return op


def _frac_affine_op():
    """One fused DVE op: out = y - round(y), y = in0*s0[p] + s1[p].

    round() is the fp32 RNE 2^23 magic (each DVE ALU slice rounds its
    fp32 result, so (y + 2^23) - 2^23 == round-to-nearest-even(y) for
    |y| < 2^22). Registered at runtime via the documented dve_ops
    extension point (append to OPS; row assigned past the last entry).
    """
    global _FRAC_OP
    if _FRAC_OP is not None:
        return _FRAC_OP
    from concourse import dve_ops
    from concourse.dve_spec import Spec, Src0, C0, C1, C2, lower, _has_src1
    from concourse.dve_uop import DveOpSpec

    name = "FRAC_AFFINE_ANT"
    for existing in dve_ops.OPS:
        if existing.name == name:
            _FRAC_OP = existing
            return existing

    y = Src0 * C0 + C1
    body = y - ((y + C2) - C2)

    def reference(in0, in1, s0, s1, imm2):
        m = np.float32(imm2)
        yv = (
            in0.astype(np.float32) * np.float32(s0) + np.float32(s1)
            if np.isscalar(s0) or np.ndim(s0) == 0
            else in0.astype(np.float32) * s0.astype(np.float32)
            + s1.astype(np.float32)
        )
        yv = yv.astype(np.float32)
        k = ((yv + m).astype(np.float32) - m).astype(np.float32)
        return (yv - k).astype(np.float32)

    spec = Spec(body=body, reference=reference)
    row = dve_ops._CUSTOM_DVE_ROW_BASE + len(dve_ops.OPS)
    assert row < 0x20, "custom-DVE row overflow"
    dve_ops._SUB_OPCODE_FOR_NAME[name] = row
    shas = {}
    for ver in ("v3", "v4"):
        s = DveOpSpec(
            name=name,
            opcode=row,
            uops=lower(spec, ver=ver),
            rd1_en=_has_src1(spec),
        )
        shas[ver] = s.sha(ver)
    op = dve_ops.DveOp(name, spec, subdim=False, uops_sha=shas)
    dve_ops.OPS.append(op)
    dve_ops.CUSTOM_DVE_SPECS[name] = spec
    _FRAC_OP = op
    return op


def _build(
    reps=1,
    mode="full",
    nsup8=None,
    dr_sw=False,
    dr_il=True,
    sg16=False,
    frac_mod=False,
    dma_fuse=6,
    ext=True,
    ext_gp=False,
    ext_st4=True,
    staggered=False,
    unroll=4,
):
    if nsup8 is None:
        nsup8 = NSUP8
    nsup16 = NSUP - nsup8
    n16 = nsup16 * SUP * P
    nc = bacc.Bacc(
        "TRN2",
        target_bir_lowering=False,
        debug=False,
        num_devices=N_CORES,
    )
    # weights are packed on the host so each partition's rows are
    # contiguous in DRAM: one 4KB/2KB descriptor per partition per DMA
    wt = nc.dram_tensor(
        "wt", [P, max(nsup16, 1) * SUP * H], F16, kind="ExternalInput"
    )
    if dr_sw:
        wt8 = nc.dram_tensor(
            "wt8", [P, max(nsup8, 1) * HT * 2 * P], F8, kind="ExternalInput"
        )
    else:
        wt8 = nc.dram_tensor(
            "wt8", [P, max(nsup8, 1) * SUP * H], F8, kind="ExternalInput"
        )
    fcol = nc.dram_tensor("fcol", [P, NCH], F32, kind="ExternalInput")
    basecol = nc.dram_tensor("basecol", [P, NCH], F32, kind="ExternalInput")
    c2col = nc.dram_tensor("c2col", [P, NCH], F32, kind="ExternalInput")
    iota = nc.dram_tensor(
        "iota", [P, F], F16 if sg16 else F32, kind="ExternalInput"
    )
    out = nc.dram_tensor("out", [H, F], F16, kind="ExternalOutput")

    with tile.TileContext(nc) as tc:
        with ExitStack() as ctx:
            _body(
                ctx,
                tc,
                wt.ap(),
                wt8.ap(),
                fcol.ap(),
                basecol.ap(),
                c2col.ap(),
                iota.ap(),
                out.ap(),
                reps,
                mode,
                nsup8=nsup8,
                dr_sw=dr_sw,
                dr_il=dr_il,
                sg16=sg16,
                frac_mod=frac_mod,
                dma_fuse=dma_fuse,
                ext=ext,
                ext_gp=ext_gp,
                ext_st4=ext_st4,
                staggered=staggered,
                unroll=unroll,
            )
    nc.compile()
    return nc


def _body(
    ctx,
    tc,
    wt_ap,
    wt8_ap,
    f_ap,
    base_ap,
    c2_ap,
    iota_ap,
    out_ap,
    reps,
    mode="full",
    nsup8=NSUP8,
    dr_sw=False,
    dr_il=False,
    sg16=False,
    frac_mod=False,
    dma_fuse=6,
    ext=True,
    ext_gp=False,
    ext_st4=True,
    staggered=False,
    unroll=4,
):
    nc = tc.nc
    nsup16 = NSUP - nsup8
    FNG = F16 if sg16 else F32  # negr / iota dtype
    const = ctx.enter_context(tc.tile_pool(name="const", bufs=1))
    wtp = ctx.enter_context(tc.tile_pool(name="wt", bufs=8))
    sp = ctx.enter_context(tc.tile_pool(name="sgen", bufs=4))
    yp = ctx.enter_context(tc.tile_pool(name="ytmp", bufs=3))
    psp = ctx.enter_context(tc.tile_pool(name="ps", bufs=1, space="PSUM"))
    op = ctx.enter_context(tc.tile_pool(name="outp", bufs=4))

    do_dma = mode in ("full", "mm_only", "dma_only")
    do_frac = mode in ("full", "sgen_only", "frac_only", "mm_nodma", "nodma_full")
    do_sin = mode in ("full", "sgen_only", "sin_only", "mm_nodma", "nodma_full")
    do_sgen = do_frac or do_sin
    do_mm = mode in ("full", "mm_only", "mm_nodma", "nodma_full")
    if mode == "mm_nodma":
        # pure-PE measurement: S generated as usual (cheap enough to hide
        # is not required -- sgen may co-run), weights are resident consts
        do_frac = do_sin = do_sgen = False
    if mode == "noop":
        do_dma = do_sgen = do_frac = do_sin = do_mm = False

    # consts go on the scalar HWDGE ring so the sync ring starts streaming
    # weights immediately
    iota_sb = const.tile([P, F], FNG, tag="iota")
    nc.scalar.dma_start(iota_sb[:], iota_ap)
    f_sb = const.tile([P, NCH], F32, tag="f")
    nc.scalar.dma_start(f_sb[:], f_ap)
    b_sb = const.tile([P, NCH], F32, tag="b")
    nc.scalar.dma_start(b_sb[:], base_ap)
    c2_sb = const.tile([P, NCH], F32, tag="c2")
    nc.scalar.dma_start(c2_sb[:], c2_ap)
    wt_const = wt8_const = None
    if mode in ("mm_nodma", "nodma_full"):
        wt_const = const.tile([P, SUP, H], F16, tag="wt_const")
        nc.vector.memset(wt_const[:], 0.01)
        wt8_const = const.tile(
            [P, HT * 2 * P] if dr_sw else [P, SUP, H], F8, tag="wt8_const"
        )
        nc.vector.memset(wt8_const[:], 0.01)
    fneg_sb = bneg_sb = None
    if frac_mod:
        # negated copies for the gpsimd FRAC path (the standard-op magic
        # round needs (t - 2^23) - yn with yn = -y to come out as +r)
        fneg_sb = const.tile([P, NCH], F32, tag="fneg")
        nc.vector.tensor_scalar_mul(fneg_sb[:], f_sb[:], -1.0)
        bneg_sb = const.tile([P, NCH], F32, tag="bneg")
        nc.vector.tensor_scalar_mul(bneg_sb[:], b_sb[:], -1.0)
    magic_col = const.tile([P, 1], F32, tag="magic_col")
    nc.vector.memset(magic_col[:], MAGIC)
    # touch Sin once so the ACT table set loads during the DMA/pipeline fill
    sin_warm = const.tile([P, 1], F32, tag="sin_warm")
    nc.scalar.activation(
        sin_warm[:],
        magic_col[:],
        mybir.ActivationFunctionType.Sin,
        scale=0.0,
    )

    fixed_s = None
    fixed_s8 = None
    if not do_sin:
        fixed_s = const.tile([P, F * SUP], F16, tag="fixed_s")
        nc.vector.memset(fixed_s[:], 0.25)
        if dr_il:
            fixed_s8 = const.tile([P, SUP * F], F8, tag="fixed_s8")
        else:
            fixed_s8 = const.tile([P, SUP, F], F8, tag="fixed_s8")
        nc.vector.memset(fixed_s8[:], 0.25)
    negr_const = None
    if do_sin and not do_frac:
        negr_const = const.tile([P, F * SUP], FNG, tag="negr_const")
        nc.vector.memset(negr_const[:], 0.123)

    ps = {}
    if do_mm:
        for h in range(HT):
            for half in range(NHALF):
                ps[(h, half)] = psp.tile(
                    [P, 512], F32, tag=f"ps{h}_{half}", name=f"ps{h}_{half}"
                )

    W = F * SUP  # free width of one fused superchunk

    def body(rep):
        if mode == "noop":
            nc.vector.memset(magic_col[:], MAGIC)
            return
        # stage functions of the software pipeline, indexed by superchunk
        wts = {}
        wtg = {}
        negr = {}
        s_t = {}

        frac_op = _frac_affine_op()
        mulsub = _mulsub_op() if ext else None
        GW = 768 if ext else F  # generated (FRAC+Sin) columns per chunk

        def st_load(sc):
            fp8 = sc >= nsup16
            if do_frac:
                negr[sc] = sp.tile([P, W], FNG, tag="negr", name=f"nr_{sc}")
            if mode in ("mm_nodma", "nodma_full"):
                wts[sc] = wt8_const if fp8 else wt_const
            elif do_dma and dma_fuse > 1:
                # fused group DMAs: one large transfer per dma_fuse
                # superchunks (fp16) / one for all fp8 superchunks,
                # alternating between the two physical HWDGE rings
                if fp8 and sc == nsup16:
                    g = (nsup16 + dma_fuse - 1) // dma_fuse
                    if dr_sw:
                        gt = wtp.tile(
                            [P, nsup8, HT * 2 * P], F8, tag="wt8g", bufs=2
                        )
                        src = wt8_ap[:].rearrange(
                            "p (g x) -> p g x", g=nsup8
                        )
                    else:
                        gt = wtp.tile(
                            [P, nsup8, SUP, H], F8, tag="wt8g", bufs=2
                        )
                        src = wt8_ap[:].rearrange(
                            "p (g s h) -> p g s h", g=nsup8, s=SUP
                        )
                    (nc.sync, nc.scalar)[g % 2].dma_start(gt[:], src)
                    wtg["fp8"] = gt
                if fp8:
                    gt = wtg["fp8"]
                    j = sc - nsup16
                    wts[sc] = gt[:, j] if dr_sw else gt[:, j]
                else:
                    if sc % dma_fuse == 0:
                        g = sc // dma_fuse
                        gs = min(dma_fuse, nsup16 - sc)
                        gt = wtp.tile(
                            [P, gs, SUP, H], F16, tag=f"wtg{gs}", bufs=2,
                            name=f"wtg_{sc}",
                        )
                        src = wt_ap[
                            :, sc * SUP * H : (sc + gs) * SUP * H
                        ].rearrange("p (g s h) -> p g s h", g=gs, s=SUP)
                        (nc.sync, nc.scalar)[g % 2].dma_start(gt[:], src)
                        wtg[g] = gt
                    wts[sc] = wtg[sc // dma_fuse][:, sc % dma_fuse]
            elif do_dma:
                # one batched DMA per superchunk: SUP chunk-rows of wt,
                # [P, SUP, H] view of DRAM -> [P, SUP, H] SBUF tile
                if fp8 and dr_sw:
                    # host pre-interleaved DoubleRowSwInterleave layout:
                    # per partition, HT blocks of 256 fp8 weights
                    wt_t = wtp.tile(
                        [P, HT * 2 * P], F8, tag="wt8", name=f"wt_{sc}"
                    )
                    base = (sc - nsup16) * HT * 2 * P
                    src = wt8_ap[:, base : base + HT * 2 * P]
                    nc.sync.dma_start(wt_t[:], src)
                    wts[sc] = wt_t
                elif fp8:
                    wt_t = wtp.tile([P, SUP, H], F8, tag="wt8", name=f"wt_{sc}")
                    base = (sc - nsup16) * SUP * H
                    src = wt8_ap[:, base : base + SUP * H].rearrange(
                        "p (s h) -> p s h", s=SUP
                    )
                    nc.sync.dma_start(wt_t[:], src)
                    wts[sc] = wt_t
                else:
                    wt_t = wtp.tile([P, SUP, H], F16, tag="wt", name=f"wt_{sc}")
                    src = wt_ap[:, sc * SUP * H : (sc + 1) * SUP * H].rearrange(
                        "p (s h) -> p s h", s=SUP
                    )
                    nc.sync.dma_start(wt_t[:], src)
                    wts[sc] = wt_t
            if do_frac:
                interleave = fp8 and dr_il and not ext
                for i in range(SUP):
                    q = sc * SUP + i
                    gw = F if fp8 else GW
                    if interleave:
                        # pair-interleaved layout (A0 B0 A1 B1 ...) so the
                        # DoubleRow rhs reads 2 fp8 per 16-bit word
                        dst = negr[sc][:].rearrange("p (l s) -> p s l", s=SUP)[
                            :, i : i + 1, :
                        ]
                    else:
                        dst = negr[sc][:, i * F : i * F + gw]
                    if frac_mod and q % 3 != 0:
                        # offload the range reduction to the (otherwise
                        # idle) gpsimd engine. The per-partition-scalar
                        # affine must stay on DVE (TensorScalarPtr is not
                        # supported on Pool), but it is one cheap standard
                        # op vs the 5-uop fused FRAC. Negated inputs so the
                        # subtraction order yields +r:
                        #   yn = -y        (DVE, AP scalars)
                        #   t = RNE(yn + 2^23)        (GP, immediate)
                        #   r = (t - 2^23) - yn = y - round(y)  (GP)
                        yn = yp.tile([P, F], F32, tag="y_gp", name=f"y_{q}")
                        nc.vector.tensor_scalar(
                            yn[:, :gw],
                            iota_sb[:, :gw],
                            fneg_sb[:, q : q + 1],
                            bneg_sb[:, q : q + 1],
                            op0=mybir.AluOpType.mult,
                            op1=mybir.AluOpType.add,
                        )
                        # k = RNE(yn + 2^23) - 2^23 = round(yn): both ALU
                        # stages round fp32, which IS the magic trick
                        t = yp.tile([P, F], F32, tag="t_gp", name=f"t_{q}")
                        nc.gpsimd.tensor_scalar(
                            t[:, :gw],
                            yn[:, :gw],
                            MAGIC,
                            -MAGIC,
                            op0=mybir.AluOpType.add,
                            op1=mybir.AluOpType.add,
                        )
                        # r = round(yn) - yn = y - round(y)
                        nc.gpsimd.tensor_sub(dst, t[:, :gw], yn[:, :gw])
                    else:
                        nc.vector._custom_dve(
                            frac_op,
                            out=dst,
                            in0=iota_sb[:, :gw],
                            s0=f_sb[:, q : q + 1],
                            s1=b_sb[:, q : q + 1],
                            imm2=MAGIC,
                        )

        def st_sin(sc):
            fp8 = sc >= nsup16
            if not do_sin:
                s_t[sc] = fixed_s8 if fp8 else fixed_s
                negr.pop(sc, None)
                return
            if fp8 and dr_il:
                s_t[sc] = sp.tile([P, SUP * F], F8, tag="s8", name=f"s_{sc}")
            elif fp8:
                s_t[sc] = sp.tile([P, SUP, F], F8, tag="s8", name=f"s_{sc}")
            else:
                s_t[sc] = sp.tile([P, W], F16, tag="s", name=f"s_{sc}")
            nsrc = negr[sc][:] if do_frac else negr_const[:]
            if ext and fp8 and dr_il:
                # FRAC wrote contiguous chunk-major; Sin emits the
                # pair-interleaved e4m3 via a strided output AP
                src = nsrc.rearrange("p (s l) -> p s l", s=SUP)
                dstv = s_t[sc][:].rearrange("p (l s) -> p s l", s=SUP)
                nc.scalar.activation(
                    dstv,
                    src,
                    mybir.ActivationFunctionType.Sin,
                    scale=SIN_SCALE,
                )
            elif ext and not fp8:
                # only the first GW columns of each chunk were range-reduced
                src = nsrc.rearrange("p (s l) -> p s l", s=SUP)[
                    :, :, :GW
                ]
                dstv = s_t[sc][:].rearrange("p (s l) -> p s l", s=SUP)[
                    :, :, :GW
                ]
                nc.scalar.activation(
                    dstv,
                    src,
                    mybir.ActivationFunctionType.Sin,
                    scale=SIN_SCALE,
                )
                if not ext_st4:
                    st_ext(sc, force=True)
            else:
                nc.scalar.activation(
                    s_t[sc][:],
                    nsrc,
                    mybir.ActivationFunctionType.Sin,
                    scale=SIN_SCALE,
                )
            negr.pop(sc, None)

        def st_ext(sc, force=False):
            # Chebyshev step fills the tail: S[m] = 2cos(Dl)S[m-D]-S[m-2D].
            # Own pipeline stage: it consumes the PREVIOUS tick's Sin, so
            # the strict-FIFO DVE queue never head-of-line blocks on ACT.
            if not (ext and do_sin) or sc >= nsup16:
                return
            if not ext_st4 and not force:
                return
            for i in range(SUP):
                q = sc * SUP + i
                if ext_gp:
                    # split: per-partition scale on DVE (4x fp16 TS),
                    # subtract on the otherwise-idle gpsimd engine
                    t = yp.tile([P, F - GW], F16, tag="extt", name=f"et_{q}")
                    nc.vector.tensor_scalar(
                        t[:],
                        s_t[sc][:, i * F + GW - 256 : i * F + GW],
                        c2_sb[:, q : q + 1],
                        0.0,
                        op0=mybir.AluOpType.mult,
                        op1=mybir.AluOpType.add,
                    )
                    nc.gpsimd.tensor_sub(
                        s_t[sc][:, i * F + GW : (i + 1) * F],
                        t[:],
                        s_t[sc][:, i * F + GW - 512 : i * F + GW - 256],
                    )
                else:
                    nc.vector._custom_dve(
                        mulsub,
                        out=s_t[sc][:, i * F + GW : (i + 1) * F],
                        in0=s_t[sc][:, i * F + GW - 256 : i * F + GW],
                        in1=s_t[sc][:, i * F + GW - 512 : i * F + GW - 256],
                        s0=c2_sb[:, q : q + 1],
                        s1=0.0,
                    )

        def st_mm(sc):
            if not do_mm:
                s_t.pop(sc, None)
                return
            fp8 = sc >= nsup16
            if fp8:
                # one DoubleRow matmul covers both chunks (K = 256)
                pm = (
                    mybir.MatmulPerfMode.DoubleRowSwInterleave
                    if dr_sw
                    else mybir.MatmulPerfMode.DoubleRow
                )
                if dr_il:
                    s_view = s_t[sc][:].rearrange("p (l s) -> p s l", s=SUP)
                else:
                    s_view = s_t[sc]
                for h in range(HT):
                    if dr_sw:
                        lhsT = wts[sc][:, h * 2 * P : (h + 1) * 2 * P]
                    else:
                        lhsT = wts[sc][:, :, h * P : (h + 1) * P]
                    for half in range(NHALF):
                        nc.tensor.matmul(
                            ps[(h, half)][:],
                            lhsT,
                            s_view[:, :, half * 512 : (half + 1) * 512],
                            start=(sc == 0),
                            stop=(sc == NSUP - 1),
                            perf_mode=pm,
                        )
            else:
                for i in range(SUP):
                    q = sc * SUP + i
                    for h in range(HT):
                        lhsT = wts[sc][:, i : i + 1, h * P : (h + 1) * P]
                        for half in range(NHALF):
                            nc.tensor.matmul(
                                ps[(h, half)][:],
                                lhsT,
                                s_t[sc][
                                    :,
                                    i * F
                                    + half * 512 : i * F
                                    + (half + 1) * 512,
                                ],
                                start=(q == 0),
                                stop=(q == NCH - 1 and nsup8 == 0),
                            )
            del wts[sc]
            s_t.pop(sc, None)

        stages = (
            [st_load, st_sin, st_ext, st_mm]
            if ext_st4
            else [st_load, st_sin, st_mm]
        )
        depth = len(stages)
        for t in range(NSUP + depth - 1):
            for si in range(depth - 1, -1, -1):
                sc = t - si
                if 0 <= sc < NSUP:
                    stages[si](sc)

    if reps == 1:
        body(0)
    else:
        n_pre = reps % unroll
        n_loop = (reps - n_pre) // unroll
        for u in range(n_pre):
            body(u)
        if n_loop > 0:
            with tc.For_i(0, n_loop, 1, staggered_reset=staggered):
                for u in range(unroll):
                    body(u)

    for h in range(HT):
        if not do_mm:
            break
        for half in range(NHALF):
            o = op.tile([P, 512], F16, tag="o", name=f"o_{h}_{half}")
            if h % 2 == 0:
                nc.scalar.activation(
                    o[:],
                    ps[(h, half)][:],
                    mybir.ActivationFunctionType.Copy,
                    scale=W_DESCALE,
                )
            else:
                nc.vector.tensor_scalar_mul(o[:], ps[(h, half)][:], W_DESCALE)
            (nc.sync, nc.scalar)[(h * NHALF + half) % 2].dma_start(
                out_ap[h * P : (h + 1) * P, half * 512 : (half + 1) * 512],
                o[:],
            )


def _prepare_inputs(W, Lambda_im, nsup8=None, dr_sw=False, sg16=False):
    if nsup8 is None:
        nsup8 = NSUP8
    n16 = (NSUP - nsup8) * SUP * P
    lam64 = np.asarray(Lambda_im, dtype=np.float64)
    f32 = (lam64 / (2 * np.pi)).astype(np.float32)
    fcol = np.ascontiguousarray(f32.reshape(NCH, P).T)
    c2 = (2.0 * np.cos(np.mod(lam64 * 256.0, 2 * np.pi))).astype(np.float32)
    c2col = np.ascontiguousarray(c2.reshape(NCH, P).T)
    nsup16 = NSUP - nsup8
    wts = np.asarray(W, dtype=np.float32)[0].T * W_PRESCALE  # [N, H]

    def pack(rows, nsup_part):
        # [nsup*SUP*P, H] -> [P, nsup*SUP*H] with each partition's rows
        # contiguous (row n = sc*SUP*P + s*P + p)
        r = rows.reshape(nsup_part, SUP, P, H).transpose(2, 0, 1, 3)
        return np.ascontiguousarray(r.reshape(P, nsup_part * SUP * H))

    if nsup16 > 0:
        wt = pack(wts[:n16].astype(np.float16), nsup16)
    else:
        wt = np.zeros((P, SUP * H), dtype=np.float16)
    w8f = np.clip(wts[n16:], -240.0, 240.0).astype(ml_dtypes.float8_e4m3)
    if dr_sw and nsup8 > 0:
        # DoubleRowSwInterleave weight layout: per superchunk sc and
        # h-tile, a 256-wide block per partition k:
        # [A(127), B(127), A(126), B(126), ..., A(0), B(0)] where
        # A/B are the h-tile columns of the two chunks of the superchunk.
        blk = w8f.reshape(nsup8, SUP, P, HT, P)  # [sc, s, k, h, m]
        inter = np.empty((nsup8, P, HT, P, SUP), dtype=w8f.dtype)
        # block position 2j holds A[127-j], 2j+1 holds B[127-j]
        inter[..., 0] = blk[:, 0][:, :, :, ::-1]
        inter[..., 1] = blk[:, 1][:, :, :, ::-1]
        wt8 = np.ascontiguousarray(
            inter.transpose(1, 0, 2, 3, 4).reshape(P, nsup8 * HT * 2 * P)
        )
    elif nsup8 > 0:
        wt8 = pack(w8f, nsup8)
    else:
        wt8 = np.zeros((P, SUP * H), dtype=ml_dtypes.float8_e4m3)
    iota = np.ascontiguousarray(
        np.broadcast_to(
            np.arange(F, dtype=np.float16 if sg16 else np.float32), (P, F)
        )
    )
    in_maps = []
    for c in range(N_CORES):
        base64 = np.mod(lam64 * (F * c) / (2 * np.pi) + 0.25, 1.0)
        basecol = np.ascontiguousarray(
            base64.astype(np.float32).reshape(NCH, P).T
        )
        in_maps.append(
            {
                "wt": wt,
                "wt8": wt8,
                "fcol": fcol,
                "basecol": basecol,
                "c2col": c2col,
                "iota": iota,
            }
        )
    return in_maps


def _run(
    W,
    Lambda_im,
    L,
    trace=False,
    reps=1,
    mode="full",
    nsup8=None,
    dr_sw=False,
    dr_il=True,
    sg16=False,
    frac_mod=False,
    dma_fuse=6,
    ext=True,
    ext_gp=False,
    ext_st4=True,
    staggered=False,
    unroll=4,
    **rbk_kwargs,
):
    assert int(L) == L_FULL, f"kernel hardcoded for L={L_FULL}, got {L}"
    key = (reps, mode, nsup8, dr_sw, dr_il, sg16, frac_mod, dma_fuse, ext,
           ext_gp, ext_st4, staggered, unroll)
    if key not in _compiled:
        _compiled[key] = _build(
            reps,
            mode,
            nsup8=nsup8,
            dr_sw=dr_sw,
            dr_il=dr_il,
            sg16=sg16,
            frac_mod=frac_mod,
            dma_fuse=dma_fuse,
            ext=ext,
            ext_gp=ext_gp,
            ext_st4=ext_st4,
            staggered=staggered,
            unroll=unroll,
        )
    nc = _compiled[key]
    in_maps = _prepare_inputs(
        W, Lambda_im, nsup8=nsup8, dr_sw=dr_sw, sg16=sg16
    )
    res = run_bass_kernel_spmd(
        nc, in_maps, list(range(N_CORES)), trace=trace, **rbk_kwargs
    )
    K = np.empty((1, H, L_FULL), dtype=np.float32)
    for c in range(N_CORES):
        K[0, :, c * F : (c + 1) * F] = res.results[c]["out"].astype(
            np.float32
        )
    return K, res


def kernel(W, Lambda_im, L):
    K, _ = _run(W, Lambda_im, L)
    return K

